# revision 33
# baseline (speedup 1.0000x reference)
"""NNUE network forward pass on 8 Trainium2 NeuronCores (Bass/Tile).

Math (per reference):
    white_ft = clip(white @ ft_w.T + ft_b, 0, 1)        # [B, 512]
    black_ft = clip(black @ ft_w.T + ft_b, 0, 1)        # [B, 512]
    x = relu(concat([white_ft, black_ft], 1) @ fc1_w.T + fc1_b)  # [B, 256]
    out = x @ fc2_w.T + fc2_b                           # [B]

Distribution: data-parallel over the batch — each of the 8 cores handles
B/8 = 512 rows end to end; weights are replicated.  No collectives.

v3 data strategy:
  * All transposes are done on the HOST: features and ft_w are
    pre-arranged P-major as [128, K/128, free] so every chunk DMA is 128
    descriptors of nsub KB contiguous bytes (the v1 DMA-xbar transposed
    loads shattered into ~256B descriptors and nearly saturated all 16
    DMA engines at ~235 GB/s, which is also the practical per-core HBM
    read ceiling).
  * Features travel as uint8 (x quantized to round(x*255)) and are
    expanded to bf16 on-chip (white on the scalar engine via
    activation(scale=1/255), black on the vector engine) — this halves
    feature DMA bytes, taking total traffic from 126MB to 84MB per core
    and moving DMA well off the critical path.  The quantization adds
    ~0.1% relative error on top of bf16's ~0.7%, far under the 2e-2 bar.
  * Feature loads ride the two HWDGE queues (sync + scalar); constants
    load via the gpsimd SWDGE queue; outputs go out on sync.
  * Warmup: chunks of 256/256/512/1024 k precede the steady 2048-k
    chunks, and the first two have dedicated buffers, so the first
    matmul waits only on ~0.5MB of DMA and the pipeline never starves.
  * The PE p-state ramps for ~3us after going busy, so a few dummy
    matmuls on the (early-loaded) fc1 weights warm it up while the
    first feature chunks are still in flight.
  * Tail: the last chunk is emitted chain-major so each (persp,
    h1-tile) PSUM chain is evicted under the next chain's matmuls; fc2
    is computed with batch as the moving dim into a [1, 512] PSUM tile,
    giving a single contiguous 2KB output DMA.

Matmul precision: bf16 inputs, fp32 PSUM accumulation.
"""

import sys

for _p in ("/opt/trn_rl_repo", "/opt/pypackages"):
    if _p not in sys.path:
        sys.path.append(_p)

import numpy as np
import ml_dtypes

import concourse.bass as bass
import concourse.mybir as mybir
import concourse.tile as tile
from concourse.bass_utils import run_bass_kernel_spmd
from concourse.vector_clock import ScopedClock

# ---------------------------------------------------------------------------
# Workaround for the pinned walrus rejecting the TileContext exit Drain when
# it carries more than one semaphore wait ("Too many sync wait commands"):
# keep one wait on the drain and move the rest onto single-wait nops that
# still precede the all-engine barrier.
# ---------------------------------------------------------------------------
_MAX_DRAIN_WAITS = 1


def _split_drain_and_barrier(self, tick_clock, wait_clock):
    nc = self.nc
    drain_inst = nc.sync.drain()
    wait_clock.add_sem_waits(
        drain_inst.ins, ScopedClock({None: tick_clock.global_clock})
    )
    si = drain_inst.ins.sync_info
    if si is not None and si.on_wait and len(si.on_wait) > _MAX_DRAIN_WAITS:
        waits = list(si.on_wait)
        drain_inst.ins.sync_info = mybir.SyncInfo(
            on_wait=waits[:_MAX_DRAIN_WAITS], on_update=list(si.on_update)
        )
        for w in waits[_MAX_DRAIN_WAITS:]:
            ni = nc.sync.nop(nofuse=True, hint="drain_wait_split")
            nsi = ni.ins.sync_info
            upd = list(nsi.on_update) if nsi is not None else []
            ni.ins.sync_info = mybir.SyncInfo(on_wait=[w], on_update=upd)

    nc.all_engine_barrier()
    assert self.sems is not None
    popped = nc._tile_sem_poison_stack.pop()
    assert popped is self._sem_poison
    # The stock exit also emits clear_and_free_semaphores (a gpsimd
    # dma_reset + sem_clear) and a second barrier: ~6us of teardown that
    # only matters if the same NEFF executes twice in one process.  This
    # kernel guarantees single execution per NEFF (kernel() rebuilds with
    # a nonce on repeat calls), so skip it.  Python-side bookkeeping
    # (freeing the sem ids for sibling tiles) is preserved.
    sem_nums = [s.num for s in self.sems.allocated().values()]
    nc._state.prepend_free_semaphores(sem_nums)
    for poison_set in nc._tile_sem_poison_stack:
        poison_set.update(sem_nums)


tile.TileContext._drain_and_barrier = _split_drain_and_barrier


def _split_multi_waits(nc, max_waits=1):
    """Walrus in this env rejects instructions with more than one sync wait.
    Split extras onto same-engine NOPs inserted immediately before (engine
    program order makes the stall equivalent)."""
    n_split = 0
    for f in nc.m.functions:
        for blk in f.blocks:
            out = []
            for ins in blk.instructions:
                si = ins.sync_info
                if si is not None and si.on_wait and len(si.on_wait) > max_waits:
                    waits = list(si.on_wait)
                    for w in waits[max_waits:]:
                        nop = mybir.InstNoOp(
                            name=f"I-{nc.next_id()}", ins=[], outs=[])
                        nop.engine = ins.engine
                        nop.sync_info = mybir.SyncInfo(
                            on_wait=[w], on_update=[])
                        out.append(nop)
                        n_split += 1
                    ins.sync_info = mybir.SyncInfo(
                        on_wait=waits[:max_waits],
                        on_update=list(si.on_update))
                out.append(ins)
            blk.instructions[:] = out
    return n_split


# ---------------------------------------------------------------------------
# Problem shapes (hardcoded per the harness contract).
# ---------------------------------------------------------------------------
BATCH = 4096
K = 40960          # feature size
H1 = 512           # feature-transform width
H2 = 256           # fc1 width
N_CORES = 8
BC = BATCH // N_CORES   # batch rows per core = 512

U8 = mybir.dt.uint8
BF16 = mybir.dt.bfloat16
F32 = mybir.dt.float32
AF = mybir.ActivationFunctionType

P = 128            # partitions
NC_K = K // P      # 320 k-subtiles total

# k-subtile counts per chunk: small warmup chunks so the first matmul
# starts after ~1MB of DMA, then steady 2048-k chunks.  The ramp is
# gentle enough that the early (slow-ish) DMA queues never underrun the
# matmul pipeline.
CHUNK_SUBS = [4, 4, 8] + [16] * 19
assert sum(CHUNK_SUBS) == NC_K

N_WARM_MM = 8      # dummy matmuls to ramp the PE p-state during DMA fill


def build_bass(n_devices=N_CORES, feat_bufs=3, nonce=0):
    n_h = H1 // P                 # 4  h1 tiles
    n_j = 2 * H1 // P             # 8  fc1 contraction tiles
    n_h2 = H2 // P                # 2  fc1 output tiles

    nc = bass.Bass("TRN2", target_bir_lowering=False, debug=False,
                   num_devices=n_devices)

    if nonce:
        # Unused input that changes the HLO hash so repeat kernel() calls
        # get a fresh NEFF (the trimmed teardown leaves semaphores dirty,
        # so a NEFF must never execute twice).
        nc.dram_tensor(f"nonce_{nonce}", [1, 1], F32, kind="ExternalInput")

    wf = nc.dram_tensor("wf", [P, NC_K, BC], U8, kind="ExternalInput")
    bl = nc.dram_tensor("bl", [P, NC_K, BC], U8, kind="ExternalInput")
    ft_wt = nc.dram_tensor("ft_wt", [P, NC_K, H1], BF16, kind="ExternalInput")
    fc1_wT = nc.dram_tensor("fc1_wT", [2 * H1, H2], BF16, kind="ExternalInput")
    # All small constants packed into one f32 tensor (one DMA):
    # cols 0:4 ft_b, 4:6 fc1_b, 6 fc2_b (replicated), 7 w2 (2 x bf16
    # bitcast per f32).
    csm = nc.dram_tensor("csm", [P, 8], F32, kind="ExternalInput")
    out = nc.dram_tensor("out", [1, BC], F32, kind="ExternalOutput")

    n_chunks = len(CHUNK_SUBS)
    with tile.TileContext(nc) as tc:
        with (
            tc.tile_pool(name="consts", bufs=1) as consts,
            tc.tile_pool(name="feats", bufs=feat_bufs) as feats,
            tc.tile_pool(name="conv", bufs=2) as conv,
            tc.tile_pool(name="warm", bufs=1) as warm,
            tc.tile_pool(name="ftout", bufs=1) as ftout,
            tc.tile_pool(name="small", bufs=1) as small,
        ):
            # --- small constants: one 4KB DMA (a single trigger), issued
            # mid-stream together with fc1w since they are only needed at
            # the eviction phase ~540us in.  The gpsimd SWDGE queue is
            # never used: any SWDGE traffic costs a ~7us ring drain at
            # NEFF exit.
            csm_sb = consts.tile([P, 8], F32, tag="csm")
            ft_b_sb = csm_sb[:, 0:n_h]
            fc1_b_sb = csm_sb[:, n_h:n_h + n_h2]
            fc2_b_sb = csm_sb[:, 6:7]
            w2_sb = csm_sb[:, 7:8].bitcast(BF16)
            # fc1w (0.5MB) is only needed near the end; its DMA is issued
            # mid-stream on the sync queue (see chunk loop below).
            fc1w_sb = consts.tile([P, n_j, H2], BF16, tag="fc1w")

            # --- stage A: feature transform ------------------------------
            # 8 PSUM accumulation chains: (perspective, h1-tile), each
            # [128 h1, 512 batch] fp32, accumulated over all of K.
            psA_cm = tc.tile_pool(name="psA", bufs=1, space="PSUM")
            psA = psA_cm.__enter__()
            pa = [
                psA.tile([P, BC], F32, tag=f"psA_{pi}_{h}", name=f"psA_{pi}_{h}")
                for pi in range(2)
                for h in range(n_h)
            ]

            # PE p-state warmup: zero-valued matmuls on a memset tile (no
            # DMA dependency, so they start ~1.5us in and the PE clock is
            # fully ramped before real data lands).  Each overwrites psA
            # chain 0 with start=stop=True; the real accumulation below
            # starts with its own start=True and resets the bank.
            warm_t = consts.tile([P, BC], BF16, tag="warm_t")
            nc.vector.memset(warm_t[:], 0.0)
            for wi in range(N_WARM_MM):
                nc.tensor.matmul(
                    pa[0][:],
                    warm_t[:, :P],
                    warm_t[:],
                    start=True, stop=True,
                )

            ft_t = [None] * (2 * n_h)

            def evict(pi, h):
                # clip(x + b, 0, 1) -> bf16 tile, doubles as fc1's
                # transposed input [j, b]
                t_relu = small.tile([P, BC], BF16, tag="relu", bufs=2,
                                    name=f"relu_{pi}_{h}")
                nc.scalar.activation(
                    t_relu[:], pa[pi * n_h + h][:], AF.Relu,
                    bias=ft_b_sb[:, h:h + 1],
                )
                t = ftout.tile([P, BC], BF16, tag=f"ft_{pi}_{h}",
                               name=f"ft_{pi}_{h}")
                nc.vector.tensor_scalar_min(t[:], t_relu[:], 1.0)
                ft_t[pi * n_h + h] = t

            s0 = 0
            for ci, n_sub in enumerate(CHUNK_SUBS):
                first = ci == 0
                last = ci == n_chunks - 1
                # uint8 feature tiles + bf16 weight tile.  The first two
                # (tiny) chunks get dedicated buffers so five chunks can
                # prefetch at t=0; steady tiles share tags and rotate.
                if ci < 2:
                    xw8 = warm.tile([P, n_sub, BC], U8, tag=f"xw8_{ci}")
                    xb8 = warm.tile([P, n_sub, BC], U8, tag=f"xb8_{ci}")
                    wt = warm.tile([P, n_sub, H1], BF16, tag=f"wt_{ci}")
                else:
                    xw8 = feats.tile([P, 16, BC], U8, tag="xw8")
                    xb8 = feats.tile([P, 16, BC], U8, tag="xb8")
                    wt = feats.tile([P, 16, H1], BF16, tag="wt")
                xw = conv.tile([P, 16, BC], BF16, tag="xw")
                xb = conv.tile([P, 16, BC], BF16, tag="xb")
                # white features + first half of the weights on the sync
                # queue, black + second half on the scalar queue: both
                # queues carry the same ~2MB per steady chunk and every
                # chunk's data arrives on both queues in subtile order.
                h1f = n_sub // 2
                nc.sync.dma_start(xw8[:, :n_sub, :], wf[:, s0:s0 + n_sub, :])
                nc.sync.dma_start(wt[:, :h1f, :], ft_wt[:, s0:s0 + h1f, :])
                nc.scalar.dma_start(xb8[:, :n_sub, :], bl[:, s0:s0 + n_sub, :])
                nc.scalar.dma_start(
                    wt[:, h1f:n_sub, :], ft_wt[:, s0 + h1f:s0 + n_sub, :])
                if ci == 14:
                    # fc1 weights + packed constants, needed from the
                    # last chunk onwards
                    nc.sync.dma_start(
                        fc1w_sb[:], fc1_wT.rearrange("(c p) n -> p c n", p=P)
                    )
                    nc.scalar.dma_start(csm_sb[:], csm[:])

                # expand uint8 -> bf16: x = q * (1/255).  The first two
                # (tiny) chunks convert per-subtile on the vector and
                # gpsimd ALUs (both idle at startup, and the scalar
                # engine's DMA-trigger queue never gates the first
                # matmul, which only waits on its own subtile); steady
                # chunks split white->scalar, black->vector.
                if ci < 2:
                    for c in range(n_sub):
                        nc.vector.tensor_scalar_mul(
                            xw[:, c:c + 1, :], xw8[:, c:c + 1, :], 1.0 / 255)
                        nc.gpsimd.tensor_scalar_mul(
                            xb[:, c:c + 1, :], xb8[:, c:c + 1, :], 1.0 / 255)
                else:
                    nc.scalar.activation(
                        xw[:, :n_sub, :], xw8[:, :n_sub, :], AF.Identity,
                        scale=1.0 / 255,
                    )
                    nc.vector.tensor_scalar_mul(
                        xb[:, :n_sub, :], xb8[:, :n_sub, :], 1.0 / 255)

                if not last:
                    for c in range(n_sub):
                        for h in range(n_h):
                            ws = wt[:, c, h * P:(h + 1) * P]
                            for pi, x in ((0, xw), (1, xb)):
                                nc.tensor.matmul(
                                    pa[pi * n_h + h][:],
                                    ws,
                                    x[:, c, :],
                                    start=first and c == 0,
                                    stop=False,
                                )
                else:
                    # chain-pair-major: finish both perspectives of one
                    # h1-tile, then evict them under the next pair's
                    # matmuls.  fc1 accumulation groups are interleaved
                    # one h-group behind the evictions, reusing the
                    # (already evicted) h=0 PSUM banks, so that after the
                    # final feature-transform matmul only one fc1 group
                    # remains.
                    pa_fc1 = [pa[0], pa[n_h]]

                    def fc1_half(j, start, stop):
                        # one contraction tile j (reads ft_t[j]) for both
                        # fc1 output tiles
                        for h2t in range(n_h2):
                            hs = slice(h2t * P, (h2t + 1) * P)
                            nc.tensor.matmul(
                                pa_fc1[h2t][:],
                                fc1w_sb[:, j, hs],
                                ft_t[j][:],
                                start=start,
                                stop=stop,
                            )

                    def fc1_group(h, start=False, stop=False):
                        fc1_half(h, start, False)
                        fc1_half(n_h + h, False, stop)

                    for h in range(n_h):
                        if h < n_h - 1:
                            for c in range(n_sub):
                                ws = wt[:, c, h * P:(h + 1) * P]
                                for pi, x in ((0, xw), (1, xb)):
                                    nc.tensor.matmul(
                                        pa[pi * n_h + h][:],
                                        ws,
                                        x[:, c, :],
                                        start=False,
                                        stop=c == n_sub - 1,
                                    )
                        else:
                            # last h-tile: run all white then all black so
                            # the white eviction is long done before fc1
                            # needs it, and only the black eviction trails
                            # the final matmul.
                            for pi, x in ((0, xw), (1, xb)):
                                for c in range(n_sub):
                                    nc.tensor.matmul(
                                        pa[pi * n_h + h][:],
                                        wt[:, c, h * P:(h + 1) * P],
                                        x[:, c, :],
                                        start=False,
                                        stop=c == n_sub - 1,
                                    )
                                evict(pi, h)
                        if h < n_h - 1:
                            evict(0, h)
                            evict(1, h)
                        if h >= 2:
                            # groups 1, 2 run under the h+1 matmuls
                            fc1_group(h - 1, start=h == 2)
                    # after the final eviction pair: group 0 (evicted long
                    # ago) first, then the white and black halves of group
                    # 3 — by the time the tensor engine reaches them the
                    # h=3 evictions have cleared, so it never stalls.
                    fc1_group(0)
                    fc1_half(n_h - 1, False, False)
                    fc1_half(2 * n_h - 1, False, True)
                s0 += n_sub

            # --- fc1 epilogue + fc2 --------------------------------------
            # x2 = relu(fc1 psum + bias): the two output tiles go to the
            # scalar and vector engines in parallel.
            x2 = []
            t2a = small.tile([P, BC], BF16, tag="x2_0", name="x2_0")
            nc.scalar.activation(
                t2a[:], pa_fc1[0][:], AF.Relu, bias=fc1_b_sb[:, 0:1]
            )
            x2.append(t2a)
            t2b = small.tile([P, BC], BF16, tag="x2_1", name="x2_1")
            nc.vector.tensor_scalar(
                t2b[:], pa_fc1[1][:], fc1_b_sb[:, 1:2], 0.0,
                mybir.AluOpType.add, mybir.AluOpType.max,
            )
            x2.append(t2b)

            # fc2: out[1, b] = sum_h2 w2[h2] * x2[h2, b] + b2.  w2 is the
            # stationary operand (a single always-ready column), and the
            # batch is processed in two halves on separate (evicted) PSUM
            # banks so the two bias-add evictions run on the scalar and
            # vector engines in parallel, meeting in one output tile.
            HB = BC // 2
            pcs = [pa[1][:1, :HB], pa[2][:1, :HB]]
            o_sb = small.tile([1, BC], F32, tag="o", name="o")
            for half in range(2):
                bs = slice(half * HB, (half + 1) * HB)
                for h2t in range(n_h2):
                    nc.tensor.matmul(
                        pcs[half],
                        w2_sb[:, h2t:h2t + 1],
                        x2[h2t][:, bs],
                        start=h2t == 0,
                        stop=h2t == n_h2 - 1,
                    )
                if half == 0:
                    nc.scalar.activation(o_sb[:1, bs], pcs[half], AF.Identity,
                                         bias=fc2_b_sb[:1, :])
                else:
                    nc.vector.tensor_scalar_add(o_sb[:1, bs], pcs[half],
                                                fc2_b_sb[:1, :])
            nc.sync.dma_start(out[:], o_sb[:])

            psA_cm.__exit__(None, None, None)

    _split_multi_waits(nc)
    return nc


# ---------------------------------------------------------------------------
# Host side
# ---------------------------------------------------------------------------
def _to_bf16(a):
    """Fast fp32 -> bf16 with round-to-nearest-even, via bit ops."""
    u = a.view(np.uint32)
    rounded = u + 0x7FFF + ((u >> 16) & 1)
    return (rounded >> 16).astype(np.uint16).view(ml_dtypes.bfloat16)


def _p_major(a, free):
    """[free_total, K] (any 1/2-byte dtype) -> contiguous [128, K//128,
    free] with out[p, c, f] = a[f, c*128 + p]."""
    v = a.reshape(free, NC_K, P)
    out = np.empty((P, NC_K, free), dtype=a.dtype)
    np.copyto(out, v.transpose(2, 1, 0))
    return out


_NC_CACHE = {"calls": 0}


def _get_nc():
    """Fresh NEFF per kernel() call beyond the first (see nonce above)."""
    n = _NC_CACHE["calls"]
    _NC_CACHE["calls"] = n + 1
    return build_bass(nonce=n), n


def _quant_u8(a):
    """fp32 in [0,1] -> uint8 round(x*255)."""
    q = np.asarray(a, np.float32) * np.float32(255.0)
    return np.rint(q, out=q).astype(np.uint8)


def kernel(white_features, black_features, ft_w, ft_b, fc1_w, fc1_b,
           fc2_w, fc2_b, **kwargs):
    nc, nonce = _get_nc()

    wf8 = _quant_u8(white_features)
    bl8 = _quant_u8(black_features)
    ft_wt = _p_major(
        _to_bf16(np.ascontiguousarray(ft_w, np.float32)).view(np.uint16), H1
    ).view(ml_dtypes.bfloat16)
    fc1_wT = _to_bf16(np.ascontiguousarray(fc1_w.T, np.float32))

    # packed small constants [128, 8] f32 (see build_bass)
    csm = np.zeros((P, 8), np.float32)
    csm[:, 0:4] = np.asarray(ft_b, np.float32).reshape(H1 // P, P).T
    csm[:, 4:6] = np.asarray(fc1_b, np.float32).reshape(H2 // P, P).T
    csm[:, 6] = np.float32(np.asarray(fc2_b, np.float32).reshape(()))
    fc2_wc = _to_bf16(np.ascontiguousarray(
        np.asarray(fc2_w, np.float32).reshape(H2 // P, P).T))
    csm[:, 7] = np.ascontiguousarray(
        fc2_wc.view(np.uint16)).view(np.float32).reshape(P)

    in_maps = []
    for c in range(N_CORES):
        rows = slice(c * BC, (c + 1) * BC)
        m = {
            "wf": _p_major(wf8[rows], BC),
            "bl": _p_major(bl8[rows], BC),
            "ft_wt": ft_wt, "fc1_wT": fc1_wT, "csm": csm,
        }
        if nonce:
            m[f"nonce_{nonce}"] = np.zeros((1, 1), np.float32)
        in_maps.append(m)

    res = run_bass_kernel_spmd(
        nc, in_maps, core_ids=list(range(N_CORES)),
        **kwargs,
    )
    full = np.concatenate(
        [res.results[c]["out"].reshape(BC) for c in range(N_CORES)])
    if kwargs:
        return full.astype(np.float32), res
    return full.astype(np.float32)
